# revision 10
# baseline (speedup 1.0000x reference)
"""Fused EmbeddingBag(mean) + Linear kernel for Trainium2, 8-core data-parallel.

Strategy: batch sharded 8 ways (2048 bags/core). The embedding table is
host-packed into bf16 "quad slots" [25002, 256]: slot s>=1 holds vocab rows
4(s-1)..4(s-1)+3, slot 0 is zeros. Token t lives in slot (t>>2)+1 at sub-row
t&3, so slot indices fit int16 — which unlocks the custom InstDMAGatherAnt
ucode (vectorized Q7 descriptor generation, ~3 ns/desc vs ~1 us fixed cost
per generic indirect DMA, which only carries one index per partition).

Q7 descriptor generation is the critical path, so bags are host-sorted by
length (descending): tile t then only needs cols_t = ceil(max_len_t/8)*8
token columns, and columns beyond cols_t are never gathered. The per-tile
column counts are baked into the compiled program (cache-keyed; a different
length profile recompiles). cols_t is monotonically non-increasing, so a
recycled gather buffer is always fully covered by what a previous tile wrote
— no stale-SBUF reads.

Per tile: ring-capacity-sized (<=1024 idx, 65 descs/lane) dma_gather chunks
round-robin 4 SWDGE queues; the idle Scalar (ACT) engine expands the
host-built bf16 mask M[p, l, j] = (j == t&3 && l < len) / max(len, 1) across
the 64 embedding lanes; the Vector engine then runs a fully contiguous bf16
multiply (2x mode) and a strided (l, j)-reduce; one matmul against
[W.T; b; null_emb] applies projection, bias, and empty-bag select. The host
un-permutes the output rows.
"""

import sys

sys.path.insert(0, "/opt/trn_rl_repo")

from contextlib import ExitStack

import numpy as np
import ml_dtypes

import concourse.bass as bass
import concourse.bacc as bacc
import concourse.mybir as mybir
import concourse.tile as tile
from concourse.bass import broadcast_tensor_aps
from concourse.masks import make_identity

VOCAB, EMBED, COND = 100000, 64, 256
B, L = 16384, 50
NCORES = 8
BLOC = B // NCORES  # 2048 bags per core
P = 128
NT = BLOC // P  # 16 tiles per core

NSLOT = VOCAB // 4 + 2  # zero slot + 25000 quad slots
QROW = 4 * EMBED  # 256 bf16 per quad slot
CHUNK_COLS = 8  # 1024 idx = 65 descs/lane; ring fits ~65-96

F32 = mybir.dt.float32
BF16 = mybir.dt.bfloat16
I16 = mybir.dt.int16

BF16_NP = ml_dtypes.bfloat16
NQUEUES = 4


def build_nc(cols: tuple) -> bass.Bass:
    """cols[t] = token columns gathered for tile t (multiple of CHUNK_COLS or
    the final partial, non-increasing, cols[t] <= L)."""
    assert len(cols) == NT and all(1 <= c <= L for c in cols)
    tot_cols = sum(cols)
    off = np.concatenate([[0], np.cumsum(cols)])  # column offsets per tile

    nc = bacc.Bacc("TRN2", target_bir_lowering=False, num_swdge_queues=NQUEUES)

    embq = nc.declare_dram_parameter("embq", [NSLOT, QROW], BF16, isOutput=False)
    idxw = nc.declare_dram_parameter("idxw", [P, tot_cols * 8], I16, isOutput=False)
    mw = nc.declare_dram_parameter("mw", [P, tot_cols * 4], BF16, isOutput=False)
    fw = nc.declare_dram_parameter("fw", [P, NT * 2], F32, isOutput=False)
    wext = nc.declare_dram_parameter("wext", [EMBED + 2, COND], F32, isOutput=False)
    out = nc.declare_dram_parameter("out", [BLOC, COND], F32, isOutput=True)

    op = mybir.AluOpType

    with tile.TileContext(nc) as tc, ExitStack() as ctx:
        const = ctx.enter_context(tc.tile_pool(name="const", bufs=1))
        sb = ctx.enter_context(tc.tile_pool(name="sb", bufs=6))
        gp = ctx.enter_context(tc.tile_pool(name="gp", bufs=2))
        mx = ctx.enter_context(tc.tile_pool(name="mx", bufs=2))
        ps = ctx.enter_context(tc.tile_pool(name="ps", bufs=2, space="PSUM"))

        # One-time constants
        idt = const.tile([P, P], F32, tag="idt")
        make_identity(nc, idt[:])
        idx_sb = const.tile([P, tot_cols * 8], I16, tag="idx")
        nc.sync.dma_start(out=idx_sb[:], in_=idxw[:])
        m_sb = const.tile([P, tot_cols * 4], BF16, tag="m")
        nc.sync.dma_start(out=m_sb[:], in_=mw[:])
        f_sb = const.tile([P, NT * 2], F32, tag="f")
        nc.sync.dma_start(out=f_sb[:], in_=fw[:])
        wext_sb = const.tile([EMBED + 2, COND], F32, tag="wext")
        nc.sync.dma_start(out=wext_sb[:], in_=wext[:])

        chunk = 0
        for t in range(NT):
            rows = slice(t * P, (t + 1) * P)
            ct = cols[t]
            ncj = ct * 4

            # Gather this tile's ct*128 quad slots in ring-sized chunks.
            gq = gp.tile([P, L * QROW], BF16, tag="gq")
            l0 = 0
            while l0 < ct:
                nsl = min(CHUNK_COLS, ct - l0)
                nidx = nsl * P
                c0 = off[t] + l0
                nc.gpsimd.dma_gather(
                    out_ap=gq[:, l0 * QROW : (l0 + nsl) * QROW].rearrange(
                        "p (l e) -> p l e", l=nsl, e=QROW
                    ),
                    in_ap=embq[:],
                    idxs_ap=idx_sb[:, c0 * 8 : (c0 + nsl) * 8],
                    num_idxs=nidx,
                    num_idxs_reg=nidx,
                    elem_size=QROW,
                    queue_num=chunk % NQUEUES,
                )
                l0 += nsl
                chunk += 1

            # ACT expands M[p, cj] across the 64 embedding lanes (stride-0
            # broadcast read, contiguous write) so the DVE multiply below
            # stays contiguous and runs in 2x bf16 mode.
            mexp = mx.tile([P, L * QROW], BF16, tag="mexp")
            m3 = m_sb[:, off[t] * 4 : off[t] * 4 + ncj].rearrange(
                "p (cj one) -> p cj one", one=1
            )
            me3 = mexp[:, : ncj * EMBED].rearrange(
                "p (cj e) -> p cj e", cj=ncj, e=EMBED
            )
            _, m3b = broadcast_tensor_aps(me3, m3)
            nc.scalar.copy(out=me3, in_=m3b)

            # Sub-row select + length mask + 1/len scaling: gq *= mexp.
            nc.vector.tensor_mul(
                out=gq[:, : ncj * EMBED],
                in0=gq[:, : ncj * EMBED],
                in1=mexp[:, : ncj * EMBED],
            )

            # mean[p, e] = sum over (l, j). A single strided reduce runs at
            # ~2.3 cyc/elem, so instead: two j-pair adds (contiguous inner
            # runs), the second writing e-major, then a contiguous l-reduce.
            g4 = gq[:].rearrange("p (l j e) -> p l j e", l=L, j=4, e=EMBED)
            nc.vector.tensor_add(
                out=g4[:, :ct, 0:2, :].rearrange("p l j e -> p l (j e)"),
                in0=g4[:, :ct, 0:2, :].rearrange("p l j e -> p l (j e)"),
                in1=g4[:, :ct, 2:4, :].rearrange("p l j e -> p l (j e)"),
            )
            t2 = sb.tile([P, EMBED * L], BF16, tag="t2")
            t2v = t2[:, : EMBED * ct].rearrange("p (e l) -> p l e", e=EMBED, l=ct)
            nc.vector.tensor_add(
                out=t2v, in0=g4[:, :ct, 0, :], in1=g4[:, :ct, 1, :]
            )
            tr = sb.tile([P, EMBED + 2], F32, tag="tr")
            nc.vector.tensor_reduce(
                out=tr[:, 0:EMBED],
                in_=t2[:, : EMBED * ct].rearrange("p (e l) -> p e l", e=EMBED, l=ct),
                axis=mybir.AxisListType.X,
                op=op.add,
            )
            nc.vector.tensor_copy(
                out=tr[:, EMBED : EMBED + 2], in_=f_sb[:, 2 * t : 2 * t + 2]
            )

            # [P, 66] -> [66, P] so the projection contracts over E on partitions
            pT = ps.tile([EMBED + 2, P], F32, tag="pT", space="PSUM")
            nc.tensor.transpose(out=pT[:], in_=tr[:], identity=idt[:])
            mT = sb.tile([EMBED + 2, P], F32, tag="mT")
            nc.scalar.copy(out=mT[:], in_=pT[:])

            # out[128, 256] = meanT.T @ [W.T; b; null]: proj + bias + null select
            po = ps.tile([P, COND], F32, tag="po", space="PSUM")
            nc.tensor.matmul(out=po[:], lhsT=mT[:], rhs=wext_sb[:], start=True, stop=True)
            ob = sb.tile([P, COND], F32, tag="ob")
            nc.scalar.copy(out=ob[:], in_=po[:])
            nc.sync.dma_start(out=out[rows, :], in_=ob[:])

    nc.compile()
    return nc


_NC_CACHE: dict = {}


def _get_nc(cols: tuple) -> bass.Bass:
    if cols not in _NC_CACHE:
        _NC_CACHE[cols] = build_nc(cols)
    return _NC_CACHE[cols]


def _pack_embq(emb_table: np.ndarray) -> np.ndarray:
    emb_bf = np.asarray(emb_table, dtype=np.float32).astype(BF16_NP)  # [V, E]
    T = np.zeros((NSLOT, QROW), dtype=BF16_NP)
    T[1 : 1 + VOCAB // 4] = emb_bf.reshape(VOCAB // 4, QROW)
    return T


def prep(token_ids, lengths, emb_table, W, b, null_emb):
    """Returns (cols, in_maps, perm). Bags are sorted by length (descending)
    within each core; perm maps sorted row -> original row."""
    ids = np.asarray(token_ids).astype(np.int64, copy=False)  # [B, L]
    lens = np.asarray(lengths).astype(np.int64, copy=False)  # [B]

    # Sort bags per core by length descending (stable for determinism).
    perm = np.concatenate(
        [
            c * BLOC + np.argsort(-lens[c * BLOC : (c + 1) * BLOC], kind="stable")
            for c in range(NCORES)
        ]
    )
    ids = ids[perm]
    lens = lens[perm]

    # Per-tile column counts, maxed across cores so one SPMD program fits all.
    lt = lens.reshape(NCORES, NT, P)
    maxlen = lt.max(axis=2).max(axis=0)  # [NT]
    cols = tuple(
        int(min(L, -(-m // CHUNK_COLS) * CHUNK_COLS)) if m > 0 else 1
        for m in np.maximum(maxlen, 1)
    )

    valid = np.arange(L)[None, :] < lens[:, None]  # [B, L]
    idx16 = np.where(valid, (ids >> 2) + 1, 0).astype(np.int16)  # [B, L]
    rec = (1.0 / np.maximum(lens, 1)).astype(np.float32)  # [B]
    sub = (ids & 3).astype(np.int64)  # [B, L]
    M = (
        (sub[:, :, None] == np.arange(4)[None, None, :]) & valid[:, :, None]
    ).astype(np.float32) * rec[:, None, None]  # [B, L, 4]
    M = M.astype(BF16_NP)
    fz = np.stack([(lens > 0), (lens == 0)], axis=1).astype(np.float32)  # [B, 2]

    embq = _pack_embq(emb_table)
    wext = np.concatenate(
        [
            np.asarray(W, dtype=np.float32).T,  # [64, 256]
            np.asarray(b, dtype=np.float32)[None, :],
            np.asarray(null_emb, dtype=np.float32)[None, :],
        ]
    )  # [66, 256]

    in_maps = []
    for c in range(NCORES):
        sl = slice(c * BLOC, (c + 1) * BLOC)
        A = idx16[sl].reshape(NT, P, L)  # [NT, P, L]
        Mc = M[sl].reshape(NT, P, L, 4)
        idx_parts, m_parts = [], []
        for t in range(NT):
            ct = cols[t]
            # idx stream: token (bag=t*128+p, l) at flat position i = l*128+p,
            # wrapped into 16 partitions (i%16, i//16), replicated to 128.
            At = A[t, :, :ct].T  # [ct, P]
            flat = At.reshape(ct * P)
            wrap = flat.reshape(ct * 8, 16).T  # [16, ct*8]
            idx_parts.append(np.tile(wrap, (8, 1)))  # [128, ct*8]
            m_parts.append(Mc[t, :, :ct, :].reshape(P, ct * 4))
        idxw = np.ascontiguousarray(np.concatenate(idx_parts, axis=1))
        mwc = np.ascontiguousarray(np.concatenate(m_parts, axis=1))
        Fc = fz[sl].reshape(NT, P, 2).transpose(1, 0, 2)
        fwc = np.ascontiguousarray(Fc.reshape(P, NT * 2))
        in_maps.append(
            {"embq": embq, "idxw": idxw, "mw": mwc, "fw": fwc, "wext": wext}
        )
    return cols, in_maps, perm


def make_in_maps(token_ids, lengths, emb_table, W, b, null_emb):
    return prep(token_ids, lengths, emb_table, W, b, null_emb)[1]


def kernel(token_ids, lengths, emb_table, W, b, null_emb, **run_kwargs):
    from concourse.bass_utils import run_bass_kernel_spmd

    cols, in_maps, perm = prep(token_ids, lengths, emb_table, W, b, null_emb)
    nc = _get_nc(cols)
    res = run_bass_kernel_spmd(nc, in_maps, core_ids=list(range(NCORES)), **run_kwargs)
    sorted_out = np.concatenate(
        [res.results[c]["out"] for c in range(NCORES)], axis=0
    )
    out = np.empty_like(sorted_out)
    out[perm] = sorted_out
    return out


# revision 11
# speedup vs baseline: 1.3546x; 1.3546x over previous
"""Fused EmbeddingBag(mean) + Linear kernel for Trainium2, 8-core data-parallel.

Strategy: batch sharded 8 ways (2048 bags/core). The embedding table is
host-packed into bf16 "quad slots" [25002, 256]: slot s>=1 holds vocab rows
4(s-1)..4(s-1)+3, slot 0 is zeros. Token t lives in slot (t>>2)+1 at sub-row
t&3, so slot indices fit int16 — which unlocks the custom InstDMAGatherAnt
ucode (vectorized Q7 descriptor generation, ~3 ns/desc vs ~1 us fixed cost
per generic indirect DMA, which only carries one index per partition).

Q7 descriptor generation is the critical path, so bags are host-sorted by
length (descending): tile t then only needs cols_t = ceil(max_len_t/8)*8
token columns, and columns beyond cols_t are never gathered. The per-tile
column counts are baked into the compiled program (cache-keyed; a different
length profile recompiles). cols_t is monotonically non-increasing, so a
recycled gather buffer is always fully covered by what a previous tile wrote
— no stale-SBUF reads.

Per tile: ring-capacity-sized (<=1024 idx, 65 descs/lane) dma_gather chunks
round-robin 4 SWDGE queues; the idle Scalar (ACT) engine expands the
host-built bf16 mask M[p, l, j] = (j == t&3 && l < len) / max(len, 1) across
the 64 embedding lanes; the Vector engine then runs a fully contiguous bf16
multiply (2x mode) and a strided (l, j)-reduce; one matmul against
[W.T; b; null_emb] applies projection, bias, and empty-bag select. The host
un-permutes the output rows.
"""

import sys

sys.path.insert(0, "/opt/trn_rl_repo")

from contextlib import ExitStack

import numpy as np
import ml_dtypes

import concourse.bass as bass
import concourse.bacc as bacc
import concourse.mybir as mybir
import concourse.tile as tile
from concourse.bass import broadcast_tensor_aps
from concourse.masks import make_identity

VOCAB, EMBED, COND = 100000, 64, 256
B, L = 16384, 50
NCORES = 8
BLOC = B // NCORES  # 2048 bags per core
P = 128
NT = BLOC // P  # 16 tiles per core

NSLOT = VOCAB // 4 + 2  # zero slot + 25000 quad slots
QROW = 4 * EMBED  # 256 bf16 per quad slot
CHUNK_COLS = 8  # 1024 idx = 65 descs/lane; ring fits ~65-96

F32 = mybir.dt.float32
BF16 = mybir.dt.bfloat16
I16 = mybir.dt.int16

BF16_NP = ml_dtypes.bfloat16
NQUEUES = 4


def build_nc(cols: tuple) -> bass.Bass:
    """cols[t] = token columns gathered for tile t (multiple of CHUNK_COLS or
    the final partial, non-increasing, cols[t] <= L)."""
    assert len(cols) == NT and all(1 <= c <= L for c in cols)
    tot_cols = sum(cols)
    off = np.concatenate([[0], np.cumsum(cols)])  # column offsets per tile

    nc = bacc.Bacc("TRN2", target_bir_lowering=False, num_swdge_queues=NQUEUES)

    embq = nc.declare_dram_parameter("embq", [NSLOT, QROW], BF16, isOutput=False)
    idxw = nc.declare_dram_parameter("idxw", [P, tot_cols * 8], I16, isOutput=False)
    mw = nc.declare_dram_parameter("mw", [P, tot_cols * 4], BF16, isOutput=False)
    fw = nc.declare_dram_parameter("fw", [P, NT * 2], F32, isOutput=False)
    wext = nc.declare_dram_parameter("wext", [EMBED + 2, COND], F32, isOutput=False)
    out = nc.declare_dram_parameter("out", [BLOC, COND], F32, isOutput=True)

    op = mybir.AluOpType

    with tile.TileContext(nc) as tc, ExitStack() as ctx:
        const = ctx.enter_context(tc.tile_pool(name="const", bufs=1))
        sb = ctx.enter_context(tc.tile_pool(name="sb", bufs=6))
        gp = ctx.enter_context(tc.tile_pool(name="gp", bufs=2))
        mx = ctx.enter_context(tc.tile_pool(name="mx", bufs=2))
        ps = ctx.enter_context(tc.tile_pool(name="ps", bufs=2, space="PSUM"))

        # One-time constants
        idt = const.tile([P, P], F32, tag="idt")
        make_identity(nc, idt[:])
        idx_sb = const.tile([P, tot_cols * 8], I16, tag="idx")
        nc.sync.dma_start(out=idx_sb[:], in_=idxw[:])
        m_sb = const.tile([P, tot_cols * 4], BF16, tag="m")
        nc.sync.dma_start(out=m_sb[:], in_=mw[:])
        f_sb = const.tile([P, NT * 2], F32, tag="f")
        nc.sync.dma_start(out=f_sb[:], in_=fw[:])
        wext_sb = const.tile([EMBED + 2, COND], F32, tag="wext")
        nc.sync.dma_start(out=wext_sb[:], in_=wext[:])

        chunk = 0
        for t in range(NT):
            rows = slice(t * P, (t + 1) * P)
            ct = cols[t]
            ncj = ct * 4

            # Gather this tile's ct*128 quad slots in ring-sized chunks.
            gq = gp.tile([P, L * QROW], BF16, tag="gq")
            l0 = 0
            while l0 < ct:
                nsl = min(CHUNK_COLS, ct - l0)
                nidx = nsl * P
                c0 = off[t] + l0
                nc.gpsimd.dma_gather(
                    out_ap=gq[:, l0 * QROW : (l0 + nsl) * QROW].rearrange(
                        "p (l e) -> p l e", l=nsl, e=QROW
                    ),
                    in_ap=embq[:],
                    idxs_ap=idx_sb[:, c0 * 8 : (c0 + nsl) * 8],
                    num_idxs=nidx,
                    num_idxs_reg=nidx,
                    elem_size=QROW,
                    queue_num=chunk % NQUEUES,
                )
                l0 += nsl
                chunk += 1

            # ACT expands M[p, cj] across the 64 embedding lanes (stride-0
            # broadcast read, contiguous write) so the DVE multiply below
            # stays contiguous and runs in 2x bf16 mode.
            mexp = mx.tile([P, L * QROW], BF16, tag="mexp")
            m3 = m_sb[:, off[t] * 4 : off[t] * 4 + ncj].rearrange(
                "p (cj one) -> p cj one", one=1
            )
            me3 = mexp[:, : ncj * EMBED].rearrange(
                "p (cj e) -> p cj e", cj=ncj, e=EMBED
            )
            _, m3b = broadcast_tensor_aps(me3, m3)
            nc.scalar.copy(out=me3, in_=m3b)

            # Sub-row select + length mask + 1/len scaling: gq *= mexp.
            nc.vector.tensor_mul(
                out=gq[:, : ncj * EMBED],
                in0=gq[:, : ncj * EMBED],
                in1=mexp[:, : ncj * EMBED],
            )

            # mean[p, e] = sum over (l, j). A single strided reduce runs at
            # ~2.3 cyc/elem, so instead: two j-pair adds (contiguous inner
            # runs), the second writing e-major, then a contiguous l-reduce.
            g4 = gq[:].rearrange("p (l j e) -> p l j e", l=L, j=4, e=EMBED)
            nc.vector.tensor_add(
                out=g4[:, :ct, 0:2, :].rearrange("p l j e -> p l (j e)"),
                in0=g4[:, :ct, 0:2, :].rearrange("p l j e -> p l (j e)"),
                in1=g4[:, :ct, 2:4, :].rearrange("p l j e -> p l (j e)"),
            )
            t2 = sb.tile([P, EMBED * L], BF16, tag="t2")
            t2v = t2[:, : EMBED * ct].rearrange("p (l e) -> p l e", l=ct, e=EMBED)
            nc.vector.tensor_add(
                out=t2v, in0=g4[:, :ct, 0, :], in1=g4[:, :ct, 1, :]
            )
            tr = sb.tile([P, EMBED + 2], F32, tag="tr")
            nc.vector.tensor_reduce(
                out=tr[:, 0:EMBED],
                in_=t2[:, : EMBED * ct].rearrange("p (l e) -> p e l", l=ct, e=EMBED),
                axis=mybir.AxisListType.X,
                op=op.add,
            )
            nc.vector.tensor_copy(
                out=tr[:, EMBED : EMBED + 2], in_=f_sb[:, 2 * t : 2 * t + 2]
            )

            # [P, 66] -> [66, P] so the projection contracts over E on partitions
            pT = ps.tile([EMBED + 2, P], F32, tag="pT", space="PSUM")
            nc.tensor.transpose(out=pT[:], in_=tr[:], identity=idt[:])
            mT = sb.tile([EMBED + 2, P], F32, tag="mT")
            nc.scalar.copy(out=mT[:], in_=pT[:])

            # out[128, 256] = meanT.T @ [W.T; b; null]: proj + bias + null select
            po = ps.tile([P, COND], F32, tag="po", space="PSUM")
            nc.tensor.matmul(out=po[:], lhsT=mT[:], rhs=wext_sb[:], start=True, stop=True)
            ob = sb.tile([P, COND], F32, tag="ob")
            nc.scalar.copy(out=ob[:], in_=po[:])
            nc.sync.dma_start(out=out[rows, :], in_=ob[:])

    nc.compile()
    return nc


_NC_CACHE: dict = {}


def _get_nc(cols: tuple) -> bass.Bass:
    if cols not in _NC_CACHE:
        _NC_CACHE[cols] = build_nc(cols)
    return _NC_CACHE[cols]


def _pack_embq(emb_table: np.ndarray) -> np.ndarray:
    emb_bf = np.asarray(emb_table, dtype=np.float32).astype(BF16_NP)  # [V, E]
    T = np.zeros((NSLOT, QROW), dtype=BF16_NP)
    T[1 : 1 + VOCAB // 4] = emb_bf.reshape(VOCAB // 4, QROW)
    return T


def prep(token_ids, lengths, emb_table, W, b, null_emb):
    """Returns (cols, in_maps, perm). Bags are sorted by length (descending)
    within each core; perm maps sorted row -> original row."""
    ids = np.asarray(token_ids).astype(np.int64, copy=False)  # [B, L]
    lens = np.asarray(lengths).astype(np.int64, copy=False)  # [B]

    # Sort bags per core by length descending (stable for determinism).
    perm = np.concatenate(
        [
            c * BLOC + np.argsort(-lens[c * BLOC : (c + 1) * BLOC], kind="stable")
            for c in range(NCORES)
        ]
    )
    ids = ids[perm]
    lens = lens[perm]

    # Per-tile column counts, maxed across cores so one SPMD program fits all.
    lt = lens.reshape(NCORES, NT, P)
    maxlen = lt.max(axis=2).max(axis=0)  # [NT]
    cols = tuple(
        int(min(L, -(-m // CHUNK_COLS) * CHUNK_COLS)) if m > 0 else 1
        for m in np.maximum(maxlen, 1)
    )

    valid = np.arange(L)[None, :] < lens[:, None]  # [B, L]
    idx16 = np.where(valid, (ids >> 2) + 1, 0).astype(np.int16)  # [B, L]
    rec = (1.0 / np.maximum(lens, 1)).astype(np.float32)  # [B]
    sub = (ids & 3).astype(np.int64)  # [B, L]
    M = (
        (sub[:, :, None] == np.arange(4)[None, None, :]) & valid[:, :, None]
    ).astype(np.float32) * rec[:, None, None]  # [B, L, 4]
    M = M.astype(BF16_NP)
    fz = np.stack([(lens > 0), (lens == 0)], axis=1).astype(np.float32)  # [B, 2]

    embq = _pack_embq(emb_table)
    wext = np.concatenate(
        [
            np.asarray(W, dtype=np.float32).T,  # [64, 256]
            np.asarray(b, dtype=np.float32)[None, :],
            np.asarray(null_emb, dtype=np.float32)[None, :],
        ]
    )  # [66, 256]

    in_maps = []
    for c in range(NCORES):
        sl = slice(c * BLOC, (c + 1) * BLOC)
        A = idx16[sl].reshape(NT, P, L)  # [NT, P, L]
        Mc = M[sl].reshape(NT, P, L, 4)
        idx_parts, m_parts = [], []
        for t in range(NT):
            ct = cols[t]
            # idx stream: token (bag=t*128+p, l) at flat position i = l*128+p,
            # wrapped into 16 partitions (i%16, i//16), replicated to 128.
            At = A[t, :, :ct].T  # [ct, P]
            flat = At.reshape(ct * P)
            wrap = flat.reshape(ct * 8, 16).T  # [16, ct*8]
            idx_parts.append(np.tile(wrap, (8, 1)))  # [128, ct*8]
            m_parts.append(Mc[t, :, :ct, :].reshape(P, ct * 4))
        idxw = np.ascontiguousarray(np.concatenate(idx_parts, axis=1))
        mwc = np.ascontiguousarray(np.concatenate(m_parts, axis=1))
        Fc = fz[sl].reshape(NT, P, 2).transpose(1, 0, 2)
        fwc = np.ascontiguousarray(Fc.reshape(P, NT * 2))
        in_maps.append(
            {"embq": embq, "idxw": idxw, "mw": mwc, "fw": fwc, "wext": wext}
        )
    return cols, in_maps, perm


def make_in_maps(token_ids, lengths, emb_table, W, b, null_emb):
    return prep(token_ids, lengths, emb_table, W, b, null_emb)[1]


def kernel(token_ids, lengths, emb_table, W, b, null_emb, **run_kwargs):
    from concourse.bass_utils import run_bass_kernel_spmd

    cols, in_maps, perm = prep(token_ids, lengths, emb_table, W, b, null_emb)
    nc = _get_nc(cols)
    res = run_bass_kernel_spmd(nc, in_maps, core_ids=list(range(NCORES)), **run_kwargs)
    sorted_out = np.concatenate(
        [res.results[c]["out"] for c in range(NCORES)], axis=0
    )
    out = np.empty_like(sorted_out)
    out[perm] = sorted_out
    return out


# revision 12
# speedup vs baseline: 1.6023x; 1.1829x over previous
"""Fused EmbeddingBag(mean) + Linear kernel for Trainium2, 8-core data-parallel.

Strategy: batch sharded 8 ways (2048 bags/core). The embedding table is
host-packed into bf16 "quad slots" [25002, 256]: slot s>=1 holds vocab rows
4(s-1)..4(s-1)+3, slot 0 is zeros. Token t lives in slot (t>>2)+1 at sub-row
t&3, so slot indices fit int16 — which unlocks the custom InstDMAGatherAnt
ucode (vectorized Q7 descriptor generation, ~3 ns/desc vs ~1 us fixed cost
per generic indirect DMA, which only carries one index per partition).

Q7 descriptor generation is the critical path, so bags are host-sorted by
length (descending): tile t then only needs cols_t = ceil(max_len_t/8)*8
token columns, and columns beyond cols_t are never gathered. The per-tile
column counts are baked into the compiled program (cache-keyed; a different
length profile recompiles). cols_t is monotonically non-increasing, so a
recycled gather buffer is always fully covered by what a previous tile wrote
— no stale-SBUF reads.

Per tile: ring-capacity-sized (<=1024 idx, 65 descs/lane) dma_gather chunks
round-robin 4 SWDGE queues; the idle Scalar (ACT) engine expands the
host-built bf16 mask M[p, l, j] = (j == t&3 && l < len) / max(len, 1) across
the 64 embedding lanes; the Vector engine then runs a fully contiguous bf16
multiply (2x mode) and a strided (l, j)-reduce; one matmul against
[W.T; b; null_emb] applies projection, bias, and empty-bag select. The host
un-permutes the output rows.
"""

import sys

sys.path.insert(0, "/opt/trn_rl_repo")

from contextlib import ExitStack

import numpy as np
import ml_dtypes

import concourse.bass as bass
import concourse.bacc as bacc
import concourse.mybir as mybir
import concourse.tile as tile
from concourse.bass import broadcast_tensor_aps
from concourse.masks import make_identity

VOCAB, EMBED, COND = 100000, 64, 256
B, L = 16384, 50
NCORES = 8
BLOC = B // NCORES  # 2048 bags per core
P = 128
NT = BLOC // P  # 16 tiles per core

NSLOT = VOCAB // 4 + 2  # zero slot + 25000 quad slots
QROW = 4 * EMBED  # 256 bf16 per quad slot
CHUNK_COLS = 8  # 1024 idx = 65 descs/lane; ring fits ~65-96

F32 = mybir.dt.float32
BF16 = mybir.dt.bfloat16
I16 = mybir.dt.int16

BF16_NP = ml_dtypes.bfloat16
NQUEUES = 4


def build_nc(cols: tuple) -> bass.Bass:
    """cols[t] = token columns gathered for tile t (multiple of CHUNK_COLS or
    the final partial, non-increasing, cols[t] <= L)."""
    assert len(cols) == NT and all(1 <= c <= L for c in cols)
    tot_cols = sum(cols)
    off = np.concatenate([[0], np.cumsum(cols)])  # column offsets per tile

    nc = bacc.Bacc("TRN2", target_bir_lowering=False, num_swdge_queues=NQUEUES)

    embq = nc.declare_dram_parameter("embq", [NSLOT, QROW], BF16, isOutput=False)
    idxw = nc.declare_dram_parameter("idxw", [P, tot_cols * 8], I16, isOutput=False)
    mw = nc.declare_dram_parameter("mw", [P, tot_cols * 4], BF16, isOutput=False)
    fw = nc.declare_dram_parameter("fw", [P, NT * 2], F32, isOutput=False)
    wext = nc.declare_dram_parameter("wext", [EMBED + 2, COND], F32, isOutput=False)
    out = nc.declare_dram_parameter("out", [BLOC, COND], F32, isOutput=True)

    op = mybir.AluOpType

    with tile.TileContext(nc) as tc, ExitStack() as ctx:
        const = ctx.enter_context(tc.tile_pool(name="const", bufs=1))
        sb = ctx.enter_context(tc.tile_pool(name="sb", bufs=4))
        gp = ctx.enter_context(tc.tile_pool(name="gp", bufs=3))
        mx = ctx.enter_context(tc.tile_pool(name="mx", bufs=2))
        ps = ctx.enter_context(tc.tile_pool(name="ps", bufs=2, space="PSUM"))

        # One-time constants
        idt = const.tile([P, P], F32, tag="idt")
        make_identity(nc, idt[:])
        idx_sb = const.tile([P, tot_cols * 8], I16, tag="idx")
        nc.sync.dma_start(out=idx_sb[:], in_=idxw[:])
        m_sb = const.tile([P, tot_cols * 4], BF16, tag="m")
        nc.sync.dma_start(out=m_sb[:], in_=mw[:])
        f_sb = const.tile([P, NT * 2], F32, tag="f")
        nc.sync.dma_start(out=f_sb[:], in_=fw[:])
        wext_sb = const.tile([EMBED + 2, COND], F32, tag="wext")
        nc.sync.dma_start(out=wext_sb[:], in_=wext[:])

        chunk = 0
        for t in range(NT):
            rows = slice(t * P, (t + 1) * P)
            ct = cols[t]
            ncj = ct * 4

            # Gather this tile's ct*128 quad slots in ring-sized chunks.
            gq = gp.tile([P, L * QROW], BF16, tag="gq")
            l0 = 0
            while l0 < ct:
                nsl = min(CHUNK_COLS, ct - l0)
                nidx = nsl * P
                c0 = off[t] + l0
                nc.gpsimd.dma_gather(
                    out_ap=gq[:, l0 * QROW : (l0 + nsl) * QROW].rearrange(
                        "p (l e) -> p l e", l=nsl, e=QROW
                    ),
                    in_ap=embq[:],
                    idxs_ap=idx_sb[:, c0 * 8 : (c0 + nsl) * 8],
                    num_idxs=nidx,
                    num_idxs_reg=nidx,
                    elem_size=QROW,
                    queue_num=chunk % NQUEUES,
                )
                l0 += nsl
                chunk += 1

            # ACT expands M[p, cj] across the 64 embedding lanes (stride-0
            # broadcast read, contiguous write) so the DVE multiply below
            # stays contiguous and runs in 2x bf16 mode.
            mexp = mx.tile([P, L * QROW], BF16, tag="mexp")
            m3 = m_sb[:, off[t] * 4 : off[t] * 4 + ncj].rearrange(
                "p (cj one) -> p cj one", one=1
            )
            me3 = mexp[:, : ncj * EMBED].rearrange(
                "p (cj e) -> p cj e", cj=ncj, e=EMBED
            )
            _, m3b = broadcast_tensor_aps(me3, m3)
            nc.scalar.copy(out=me3, in_=m3b)

            # Sub-row select + length mask + 1/len scaling: gq *= mexp.
            nc.vector.tensor_mul(
                out=gq[:, : ncj * EMBED],
                in0=gq[:, : ncj * EMBED],
                in1=mexp[:, : ncj * EMBED],
            )

            # mean[p, e] = sum over (l, j). A single strided reduce runs at
            # ~2.3 cyc/elem, so instead: two j-pair adds (contiguous inner
            # runs), the second writing e-major, then a contiguous l-reduce.
            g4 = gq[:].rearrange("p (l j e) -> p l j e", l=L, j=4, e=EMBED)
            nc.vector.tensor_add(
                out=g4[:, :ct, 0:2, :].rearrange("p l j e -> p l (j e)"),
                in0=g4[:, :ct, 0:2, :].rearrange("p l j e -> p l (j e)"),
                in1=g4[:, :ct, 2:4, :].rearrange("p l j e -> p l (j e)"),
            )
            t2 = sb.tile([P, EMBED * L], BF16, tag="t2")
            t2v = t2[:, : EMBED * ct].rearrange("p (l e) -> p l e", l=ct, e=EMBED)
            nc.vector.tensor_add(
                out=t2v, in0=g4[:, :ct, 0, :], in1=g4[:, :ct, 1, :]
            )
            tr = sb.tile([P, EMBED + 2], F32, tag="tr")
            nc.vector.tensor_reduce(
                out=tr[:, 0:EMBED],
                in_=t2[:, : EMBED * ct].rearrange("p (l e) -> p e l", l=ct, e=EMBED),
                axis=mybir.AxisListType.X,
                op=op.add,
            )
            nc.vector.tensor_copy(
                out=tr[:, EMBED : EMBED + 2], in_=f_sb[:, 2 * t : 2 * t + 2]
            )

            # [P, 66] -> [66, P] so the projection contracts over E on partitions
            pT = ps.tile([EMBED + 2, P], F32, tag="pT", space="PSUM")
            nc.tensor.transpose(out=pT[:], in_=tr[:], identity=idt[:])
            mT = sb.tile([EMBED + 2, P], F32, tag="mT")
            nc.scalar.copy(out=mT[:], in_=pT[:])

            # out[128, 256] = meanT.T @ [W.T; b; null]: proj + bias + null select
            po = ps.tile([P, COND], F32, tag="po", space="PSUM")
            nc.tensor.matmul(out=po[:], lhsT=mT[:], rhs=wext_sb[:], start=True, stop=True)
            ob = sb.tile([P, COND], F32, tag="ob")
            nc.scalar.copy(out=ob[:], in_=po[:])
            nc.sync.dma_start(out=out[rows, :], in_=ob[:])

    nc.compile()
    return nc


_NC_CACHE: dict = {}


def _get_nc(cols: tuple) -> bass.Bass:
    if cols not in _NC_CACHE:
        _NC_CACHE[cols] = build_nc(cols)
    return _NC_CACHE[cols]


def _pack_embq(emb_table: np.ndarray) -> np.ndarray:
    emb_bf = np.asarray(emb_table, dtype=np.float32).astype(BF16_NP)  # [V, E]
    T = np.zeros((NSLOT, QROW), dtype=BF16_NP)
    T[1 : 1 + VOCAB // 4] = emb_bf.reshape(VOCAB // 4, QROW)
    return T


def prep(token_ids, lengths, emb_table, W, b, null_emb):
    """Returns (cols, in_maps, perm). Bags are sorted by length (descending)
    within each core; perm maps sorted row -> original row."""
    ids = np.asarray(token_ids).astype(np.int64, copy=False)  # [B, L]
    lens = np.asarray(lengths).astype(np.int64, copy=False)  # [B]

    # Sort bags per core by length descending (stable for determinism).
    perm = np.concatenate(
        [
            c * BLOC + np.argsort(-lens[c * BLOC : (c + 1) * BLOC], kind="stable")
            for c in range(NCORES)
        ]
    )
    ids = ids[perm]
    lens = lens[perm]

    # Per-tile column counts, maxed across cores so one SPMD program fits all.
    lt = lens.reshape(NCORES, NT, P)
    maxlen = lt.max(axis=2).max(axis=0)  # [NT]
    cols = tuple(
        int(min(L, -(-m // CHUNK_COLS) * CHUNK_COLS)) if m > 0 else 1
        for m in np.maximum(maxlen, 1)
    )

    valid = np.arange(L)[None, :] < lens[:, None]  # [B, L]
    idx16 = np.where(valid, (ids >> 2) + 1, 0).astype(np.int16)  # [B, L]
    rec = (1.0 / np.maximum(lens, 1)).astype(np.float32)  # [B]
    sub = (ids & 3).astype(np.int64)  # [B, L]
    M = (
        (sub[:, :, None] == np.arange(4)[None, None, :]) & valid[:, :, None]
    ).astype(np.float32) * rec[:, None, None]  # [B, L, 4]
    M = M.astype(BF16_NP)
    fz = np.stack([(lens > 0), (lens == 0)], axis=1).astype(np.float32)  # [B, 2]

    embq = _pack_embq(emb_table)
    wext = np.concatenate(
        [
            np.asarray(W, dtype=np.float32).T,  # [64, 256]
            np.asarray(b, dtype=np.float32)[None, :],
            np.asarray(null_emb, dtype=np.float32)[None, :],
        ]
    )  # [66, 256]

    in_maps = []
    for c in range(NCORES):
        sl = slice(c * BLOC, (c + 1) * BLOC)
        A = idx16[sl].reshape(NT, P, L)  # [NT, P, L]
        Mc = M[sl].reshape(NT, P, L, 4)
        idx_parts, m_parts = [], []
        for t in range(NT):
            ct = cols[t]
            # idx stream: token (bag=t*128+p, l) at flat position i = l*128+p,
            # wrapped into 16 partitions (i%16, i//16), replicated to 128.
            At = A[t, :, :ct].T  # [ct, P]
            flat = At.reshape(ct * P)
            wrap = flat.reshape(ct * 8, 16).T  # [16, ct*8]
            idx_parts.append(np.tile(wrap, (8, 1)))  # [128, ct*8]
            m_parts.append(Mc[t, :, :ct, :].reshape(P, ct * 4))
        idxw = np.ascontiguousarray(np.concatenate(idx_parts, axis=1))
        mwc = np.ascontiguousarray(np.concatenate(m_parts, axis=1))
        Fc = fz[sl].reshape(NT, P, 2).transpose(1, 0, 2)
        fwc = np.ascontiguousarray(Fc.reshape(P, NT * 2))
        in_maps.append(
            {"embq": embq, "idxw": idxw, "mw": mwc, "fw": fwc, "wext": wext}
        )
    return cols, in_maps, perm


def make_in_maps(token_ids, lengths, emb_table, W, b, null_emb):
    return prep(token_ids, lengths, emb_table, W, b, null_emb)[1]


def kernel(token_ids, lengths, emb_table, W, b, null_emb, **run_kwargs):
    from concourse.bass_utils import run_bass_kernel_spmd

    cols, in_maps, perm = prep(token_ids, lengths, emb_table, W, b, null_emb)
    nc = _get_nc(cols)
    res = run_bass_kernel_spmd(nc, in_maps, core_ids=list(range(NCORES)), **run_kwargs)
    sorted_out = np.concatenate(
        [res.results[c]["out"] for c in range(NCORES)], axis=0
    )
    out = np.empty_like(sorted_out)
    out[perm] = sorted_out
    return out


# revision 13
# speedup vs baseline: 1.9535x; 1.2192x over previous
"""Fused EmbeddingBag(mean) + Linear kernel for Trainium2, 8-core data-parallel.

Strategy: batch sharded 8 ways (2048 bags/core). The embedding table is
host-packed into bf16 "quad slots" [25002, 256]: slot s>=1 holds vocab rows
4(s-1)..4(s-1)+3, slot 0 is zeros. Token t lives in slot (t>>2)+1 at sub-row
t&3, so slot indices fit int16 — which unlocks the custom InstDMAGatherAnt
ucode (vectorized Q7 descriptor generation, ~3 ns/desc vs ~1 us fixed cost
per generic indirect DMA, which only carries one index per partition).

Q7 descriptor generation is the critical path, so bags are host-sorted by
length (descending): tile t then only needs cols_t = ceil(max_len_t/8)*8
token columns, and columns beyond cols_t are never gathered. The per-tile
column counts are baked into the compiled program (cache-keyed; a different
length profile recompiles). cols_t is monotonically non-increasing, so a
recycled gather buffer is always fully covered by what a previous tile wrote
— no stale-SBUF reads.

Per tile: ring-capacity-sized (<=1024 idx, 65 descs/lane) dma_gather chunks
round-robin 4 SWDGE queues; the idle Scalar (ACT) engine expands the
host-built bf16 mask M[p, l, j] = (j == t&3 && l < len) / max(len, 1) across
the 64 embedding lanes; the Vector engine then runs a fully contiguous bf16
multiply (2x mode) and a strided (l, j)-reduce; one matmul against
[W.T; b; null_emb] applies projection, bias, and empty-bag select. The host
un-permutes the output rows.
"""

import sys

sys.path.insert(0, "/opt/trn_rl_repo")

from contextlib import ExitStack

import numpy as np
import ml_dtypes

import concourse.bass as bass
import concourse.bacc as bacc
import concourse.mybir as mybir
import concourse.tile as tile
from concourse.bass import broadcast_tensor_aps
from concourse.masks import make_identity

VOCAB, EMBED, COND = 100000, 64, 256
B, L = 16384, 50
NCORES = 8
BLOC = B // NCORES  # 2048 bags per core
P = 128
NT = BLOC // P  # 16 tiles per core

NSLOT = VOCAB // 4 + 2  # zero slot + 25000 quad slots
QROW = 4 * EMBED  # 256 bf16 per quad slot
CHUNK_COLS = 8  # 1024 idx = 65 descs/lane; ring fits ~65-96

F32 = mybir.dt.float32
BF16 = mybir.dt.bfloat16
I16 = mybir.dt.int16

BF16_NP = ml_dtypes.bfloat16
NQUEUES = 4


def build_nc(cols: tuple) -> bass.Bass:
    """cols[t] = token columns gathered for tile t (multiple of CHUNK_COLS or
    the final partial, non-increasing, cols[t] <= L)."""
    assert len(cols) == NT and all(1 <= c <= L for c in cols)
    tot_cols = sum(cols)
    off = np.concatenate([[0], np.cumsum(cols)])  # column offsets per tile

    nc = bacc.Bacc("TRN2", target_bir_lowering=False, num_swdge_queues=NQUEUES)

    embq = nc.declare_dram_parameter("embq", [NSLOT, QROW], BF16, isOutput=False)
    idxw = nc.declare_dram_parameter("idxw", [P, tot_cols * 8], I16, isOutput=False)
    mw = nc.declare_dram_parameter("mw", [P, tot_cols * 4], BF16, isOutput=False)
    fw = nc.declare_dram_parameter("fw", [P, NT * 2], F32, isOutput=False)
    wext = nc.declare_dram_parameter("wext", [EMBED + 2, COND], F32, isOutput=False)
    out = nc.declare_dram_parameter("out", [BLOC, COND], F32, isOutput=True)

    op = mybir.AluOpType

    with tile.TileContext(nc) as tc, ExitStack() as ctx:
        const = ctx.enter_context(tc.tile_pool(name="const", bufs=1))
        sb = ctx.enter_context(tc.tile_pool(name="sb", bufs=4))
        gp = ctx.enter_context(tc.tile_pool(name="gp", bufs=3))
        mx = ctx.enter_context(tc.tile_pool(name="mx", bufs=2))
        ps = ctx.enter_context(tc.tile_pool(name="ps", bufs=2, space="PSUM"))

        # One-time constants
        idt = const.tile([P, P], F32, tag="idt")
        make_identity(nc, idt[:])
        idx_sb = const.tile([P, tot_cols * 8], I16, tag="idx")
        nc.sync.dma_start(out=idx_sb[:], in_=idxw[:])
        m_sb = const.tile([P, tot_cols * 4], BF16, tag="m")
        nc.sync.dma_start(out=m_sb[:], in_=mw[:])
        f_sb = const.tile([P, NT * 2], F32, tag="f")
        nc.sync.dma_start(out=f_sb[:], in_=fw[:])
        wext_sb = const.tile([EMBED + 2, COND], F32, tag="wext")
        nc.sync.dma_start(out=wext_sb[:], in_=wext[:])

        chunk = 0
        for t in range(NT):
            rows = slice(t * P, (t + 1) * P)
            ct = cols[t]
            ncj = ct * 4

            # Gather this tile's ct*128 quad slots in ring-sized chunks.
            gq = gp.tile([P, L * QROW], BF16, tag="gq")
            l0 = 0
            while l0 < ct:
                nsl = min(CHUNK_COLS, ct - l0)
                nidx = nsl * P
                c0 = off[t] + l0
                nc.gpsimd.dma_gather(
                    out_ap=gq[:, l0 * QROW : (l0 + nsl) * QROW].rearrange(
                        "p (l e) -> p l e", l=nsl, e=QROW
                    ),
                    in_ap=embq[:],
                    idxs_ap=idx_sb[:, c0 * 8 : (c0 + nsl) * 8],
                    num_idxs=nidx,
                    num_idxs_reg=nidx,
                    elem_size=QROW,
                    queue_num=chunk % NQUEUES,
                )
                l0 += nsl
                chunk += 1

            # ACT expands M[p, cj] across the 64 embedding lanes (stride-0
            # broadcast read, contiguous write) so the DVE multiply below
            # stays contiguous and runs in 2x bf16 mode.
            mexp = mx.tile([P, L * QROW], BF16, tag="mexp")
            m3 = m_sb[:, off[t] * 4 : off[t] * 4 + ncj].rearrange(
                "p (cj one) -> p cj one", one=1
            )
            me3 = mexp[:, : ncj * EMBED].rearrange(
                "p (cj e) -> p cj e", cj=ncj, e=EMBED
            )
            _, m3b = broadcast_tensor_aps(me3, m3)
            nc.scalar.copy(out=me3, in_=m3b)

            # Sub-row select + length mask + 1/len scaling: gq *= mexp.
            nc.vector.tensor_mul(
                out=gq[:, : ncj * EMBED],
                in0=gq[:, : ncj * EMBED],
                in1=mexp[:, : ncj * EMBED],
            )

            # mean[p, e] = sum over (l, j). A single strided reduce runs at
            # ~2.3 cyc/elem, so instead: two j-pair adds (contiguous inner
            # runs), the second writing e-major, then a contiguous l-reduce.
            g4 = gq[:].rearrange("p (l j e) -> p l j e", l=L, j=4, e=EMBED)
            nc.vector.tensor_add(
                out=g4[:, :ct, 0:2, :].rearrange("p l j e -> p l (j e)"),
                in0=g4[:, :ct, 0:2, :].rearrange("p l j e -> p l (j e)"),
                in1=g4[:, :ct, 2:4, :].rearrange("p l j e -> p l (j e)"),
            )
            t2 = sb.tile([P, EMBED * L], BF16, tag="t2")
            t2v = t2[:, : EMBED * ct].rearrange("p (l e) -> p l e", l=ct, e=EMBED)
            nc.vector.tensor_add(
                out=t2v, in0=g4[:, :ct, 0, :], in1=g4[:, :ct, 1, :]
            )
            tr = sb.tile([P, EMBED + 2], F32, tag="tr")
            nc.vector.tensor_reduce(
                out=tr[:, 0:EMBED],
                in_=t2[:, : EMBED * ct].rearrange("p (l e) -> p e l", l=ct, e=EMBED),
                axis=mybir.AxisListType.X,
                op=op.add,
            )
            nc.vector.tensor_copy(
                out=tr[:, EMBED : EMBED + 2], in_=f_sb[:, 2 * t : 2 * t + 2]
            )

            # [P, 66] -> [66, P] so the projection contracts over E on partitions
            pT = ps.tile([EMBED + 2, P], F32, tag="pT", space="PSUM")
            nc.tensor.transpose(out=pT[:], in_=tr[:], identity=idt[:])
            mT = sb.tile([EMBED + 2, P], F32, tag="mT")
            nc.scalar.copy(out=mT[:], in_=pT[:])

            # out[128, 256] = meanT.T @ [W.T; b; null]: proj + bias + null select
            po = ps.tile([P, COND], F32, tag="po", space="PSUM")
            nc.tensor.matmul(out=po[:], lhsT=mT[:], rhs=wext_sb[:], start=True, stop=True)
            ob = sb.tile([P, COND], F32, tag="ob")
            nc.scalar.copy(out=ob[:], in_=po[:])
            nc.sync.dma_start(out=out[rows, :], in_=ob[:])

    nc.compile()
    return nc


_NC_CACHE: dict = {}


def _get_nc(cols: tuple) -> bass.Bass:
    if cols not in _NC_CACHE:
        _NC_CACHE[cols] = build_nc(cols)
    return _NC_CACHE[cols]


def _pack_embq(emb_table: np.ndarray) -> np.ndarray:
    emb_bf = np.asarray(emb_table, dtype=np.float32).astype(BF16_NP)  # [V, E]
    T = np.zeros((NSLOT, QROW), dtype=BF16_NP)
    T[1 : 1 + VOCAB // 4] = emb_bf.reshape(VOCAB // 4, QROW)
    return T


def prep(token_ids, lengths, emb_table, W, b, null_emb):
    """Returns (cols, in_maps, perm). Bags are sorted by length (descending)
    within each core; perm maps sorted row -> original row."""
    ids = np.asarray(token_ids).astype(np.int64, copy=False)  # [B, L]
    lens = np.asarray(lengths).astype(np.int64, copy=False)  # [B]

    # Sort bags per core by length descending (stable for determinism).
    perm = np.concatenate(
        [
            c * BLOC + np.argsort(-lens[c * BLOC : (c + 1) * BLOC], kind="stable")
            for c in range(NCORES)
        ]
    )
    ids = ids[perm]
    lens = lens[perm]

    # Per-tile column counts, maxed across cores so one SPMD program fits all.
    lt = lens.reshape(NCORES, NT, P)
    maxlen = lt.max(axis=2).max(axis=0)  # [NT]
    cols = tuple(int(m) for m in np.maximum(maxlen, 1))

    valid = np.arange(L)[None, :] < lens[:, None]  # [B, L]
    idx16 = np.where(valid, (ids >> 2) + 1, 0).astype(np.int16)  # [B, L]
    rec = (1.0 / np.maximum(lens, 1)).astype(np.float32)  # [B]
    sub = (ids & 3).astype(np.int64)  # [B, L]
    M = (
        (sub[:, :, None] == np.arange(4)[None, None, :]) & valid[:, :, None]
    ).astype(np.float32) * rec[:, None, None]  # [B, L, 4]
    M = M.astype(BF16_NP)
    fz = np.stack([(lens > 0), (lens == 0)], axis=1).astype(np.float32)  # [B, 2]

    embq = _pack_embq(emb_table)
    wext = np.concatenate(
        [
            np.asarray(W, dtype=np.float32).T,  # [64, 256]
            np.asarray(b, dtype=np.float32)[None, :],
            np.asarray(null_emb, dtype=np.float32)[None, :],
        ]
    )  # [66, 256]

    in_maps = []
    for c in range(NCORES):
        sl = slice(c * BLOC, (c + 1) * BLOC)
        A = idx16[sl].reshape(NT, P, L)  # [NT, P, L]
        Mc = M[sl].reshape(NT, P, L, 4)
        idx_parts, m_parts = [], []
        for t in range(NT):
            ct = cols[t]
            # idx stream: token (bag=t*128+p, l) at flat position i = l*128+p,
            # wrapped into 16 partitions (i%16, i//16), replicated to 128.
            At = A[t, :, :ct].T  # [ct, P]
            flat = At.reshape(ct * P)
            wrap = flat.reshape(ct * 8, 16).T  # [16, ct*8]
            idx_parts.append(np.tile(wrap, (8, 1)))  # [128, ct*8]
            m_parts.append(Mc[t, :, :ct, :].reshape(P, ct * 4))
        idxw = np.ascontiguousarray(np.concatenate(idx_parts, axis=1))
        mwc = np.ascontiguousarray(np.concatenate(m_parts, axis=1))
        Fc = fz[sl].reshape(NT, P, 2).transpose(1, 0, 2)
        fwc = np.ascontiguousarray(Fc.reshape(P, NT * 2))
        in_maps.append(
            {"embq": embq, "idxw": idxw, "mw": mwc, "fw": fwc, "wext": wext}
        )
    return cols, in_maps, perm


def make_in_maps(token_ids, lengths, emb_table, W, b, null_emb):
    return prep(token_ids, lengths, emb_table, W, b, null_emb)[1]


def kernel(token_ids, lengths, emb_table, W, b, null_emb, **run_kwargs):
    from concourse.bass_utils import run_bass_kernel_spmd

    cols, in_maps, perm = prep(token_ids, lengths, emb_table, W, b, null_emb)
    nc = _get_nc(cols)
    res = run_bass_kernel_spmd(nc, in_maps, core_ids=list(range(NCORES)), **run_kwargs)
    sorted_out = np.concatenate(
        [res.results[c]["out"] for c in range(NCORES)], axis=0
    )
    out = np.empty_like(sorted_out)
    out[perm] = sorted_out
    return out


# revision 15
# speedup vs baseline: 2.0646x; 1.0569x over previous
"""Fused EmbeddingBag(mean) + Linear kernel for Trainium2, 8-core data-parallel.

Strategy: batch sharded 8 ways (2048 bags/core). The embedding table is
host-packed into bf16 "quad slots" [25002, 256]: slot s>=1 holds vocab rows
4(s-1)..4(s-1)+3, slot 0 is zeros. Token t lives in slot (t>>2)+1 at sub-row
t&3, so slot indices fit int16 — which unlocks the custom InstDMAGatherAnt
ucode (vectorized Q7 descriptor generation, ~3 ns/desc vs ~1 us fixed cost
per generic indirect DMA, which only carries one index per partition).

Q7 descriptor generation is the critical path, so bags are host-sorted by
length (descending): tile t then only needs cols_t = ceil(max_len_t/8)*8
token columns, and columns beyond cols_t are never gathered. The per-tile
column counts are baked into the compiled program (cache-keyed; a different
length profile recompiles). cols_t is monotonically non-increasing, so a
recycled gather buffer is always fully covered by what a previous tile wrote
— no stale-SBUF reads.

Per tile: ring-capacity-sized (<=1024 idx, 65 descs/lane) dma_gather chunks
round-robin 4 SWDGE queues; the idle Scalar (ACT) engine expands the
host-built bf16 mask M[p, l, j] = (j == t&3 && l < len) / max(len, 1) across
the 64 embedding lanes; the Vector engine then runs a fully contiguous bf16
multiply (2x mode) and a strided (l, j)-reduce; one matmul against
[W.T; b; null_emb] applies projection, bias, and empty-bag select. The host
un-permutes the output rows.
"""

import sys

sys.path.insert(0, "/opt/trn_rl_repo")

from contextlib import ExitStack

import numpy as np
import ml_dtypes

import concourse.bass as bass
import concourse.bacc as bacc
import concourse.mybir as mybir
import concourse.tile as tile
from concourse.bass import broadcast_tensor_aps
from concourse.masks import make_identity

VOCAB, EMBED, COND = 100000, 64, 256
B, L = 16384, 50
NCORES = 8
BLOC = B // NCORES  # 2048 bags per core
P = 128
NT = BLOC // P  # 16 tiles per core

NSLOT = VOCAB // 4 + 2  # zero slot + 25000 quad slots
QROW = 4 * EMBED  # 256 bf16 per quad slot
CHUNK_COLS = 8  # 1024 idx = 65 descs/lane; ring fits ~65-96

F32 = mybir.dt.float32
BF16 = mybir.dt.bfloat16
I16 = mybir.dt.int16

BF16_NP = ml_dtypes.bfloat16
NQUEUES = 4


def build_nc(cols: tuple) -> bass.Bass:
    """cols[t] = token columns gathered for tile t (multiple of CHUNK_COLS or
    the final partial, non-increasing, cols[t] <= L)."""
    assert len(cols) == NT and all(1 <= c <= L for c in cols)
    tot_cols = sum(cols)
    off = np.concatenate([[0], np.cumsum(cols)])  # column offsets per tile

    nc = bacc.Bacc("TRN2", target_bir_lowering=False, num_swdge_queues=NQUEUES)

    embq = nc.declare_dram_parameter("embq", [NSLOT, QROW], BF16, isOutput=False)
    idxw = nc.declare_dram_parameter("idxw", [P, tot_cols * 8], I16, isOutput=False)
    mw = nc.declare_dram_parameter("mw", [P, tot_cols * 4], BF16, isOutput=False)
    fw = nc.declare_dram_parameter("fw", [P, NT * 2], F32, isOutput=False)
    wext = nc.declare_dram_parameter("wext", [EMBED + 2, COND], F32, isOutput=False)
    out = nc.declare_dram_parameter("out", [BLOC, COND], F32, isOutput=True)

    op = mybir.AluOpType

    with tile.TileContext(nc) as tc, ExitStack() as ctx:
        const = ctx.enter_context(tc.tile_pool(name="const", bufs=1))
        sb = ctx.enter_context(tc.tile_pool(name="sb", bufs=4))
        gp = ctx.enter_context(tc.tile_pool(name="gp", bufs=3))
        mx = ctx.enter_context(tc.tile_pool(name="mx", bufs=2))
        ps = ctx.enter_context(tc.tile_pool(name="ps", bufs=2, space="PSUM"))

        # One-time constants
        idt = const.tile([P, P], F32, tag="idt")
        make_identity(nc, idt[:])
        # Per-tile slices so tile 0's gather doesn't wait for the full load.
        idx_sb = const.tile([P, tot_cols * 8], I16, tag="idx")
        m_sb = const.tile([P, tot_cols * 4], BF16, tag="m")
        for t in range(NT):
            c0, c1 = off[t], off[t + 1]
            nc.sync.dma_start(
                out=idx_sb[:, c0 * 8 : c1 * 8], in_=idxw[:, c0 * 8 : c1 * 8]
            )
            nc.sync.dma_start(
                out=m_sb[:, c0 * 4 : c1 * 4], in_=mw[:, c0 * 4 : c1 * 4]
            )
        f_sb = const.tile([P, NT * 2], F32, tag="f")
        nc.sync.dma_start(out=f_sb[:], in_=fw[:])
        wext_sb = const.tile([EMBED + 2, COND], F32, tag="wext")
        nc.sync.dma_start(out=wext_sb[:], in_=wext[:])

        chunk = 0
        for t in range(NT):
            rows = slice(t * P, (t + 1) * P)
            ct = cols[t]
            ncj = ct * 4

            # Gather this tile's ct*128 quad slots in ring-sized chunks.
            gq = gp.tile([P, L * QROW], BF16, tag="gq")
            l0 = 0
            while l0 < ct:
                nsl = min(CHUNK_COLS, ct - l0)
                nidx = nsl * P
                c0 = off[t] + l0
                nc.gpsimd.dma_gather(
                    out_ap=gq[:, l0 * QROW : (l0 + nsl) * QROW].rearrange(
                        "p (l e) -> p l e", l=nsl, e=QROW
                    ),
                    in_ap=embq[:],
                    idxs_ap=idx_sb[:, c0 * 8 : (c0 + nsl) * 8],
                    num_idxs=nidx,
                    num_idxs_reg=nidx,
                    elem_size=QROW,
                    queue_num=chunk % NQUEUES,
                )
                l0 += nsl
                chunk += 1

            # ACT expands M[p, cj] across the 64 embedding lanes (stride-0
            # broadcast read, contiguous write) so the DVE multiply below
            # stays contiguous and runs in 2x bf16 mode.
            mexp = mx.tile([P, L * QROW], BF16, tag="mexp")
            m3 = m_sb[:, off[t] * 4 : off[t] * 4 + ncj].rearrange(
                "p (cj one) -> p cj one", one=1
            )
            me3 = mexp[:, : ncj * EMBED].rearrange(
                "p (cj e) -> p cj e", cj=ncj, e=EMBED
            )
            _, m3b = broadcast_tensor_aps(me3, m3)
            nc.scalar.copy(out=me3, in_=m3b)

            # Sub-row select + length mask + 1/len scaling: gq *= mexp.
            nc.vector.tensor_mul(
                out=gq[:, : ncj * EMBED],
                in0=gq[:, : ncj * EMBED],
                in1=mexp[:, : ncj * EMBED],
            )

            # mean[p, e] = sum over (l, j). A single strided reduce runs at
            # ~2.3 cyc/elem, so instead: two j-pair adds (contiguous inner
            # runs), the second writing e-major, then a contiguous l-reduce.
            g4 = gq[:].rearrange("p (l j e) -> p l j e", l=L, j=4, e=EMBED)
            nc.vector.tensor_add(
                out=g4[:, :ct, 0:2, :].rearrange("p l j e -> p l (j e)"),
                in0=g4[:, :ct, 0:2, :].rearrange("p l j e -> p l (j e)"),
                in1=g4[:, :ct, 2:4, :].rearrange("p l j e -> p l (j e)"),
            )
            t2 = sb.tile([P, EMBED * L], BF16, tag="t2")
            t2v = t2[:, : EMBED * ct].rearrange("p (l e) -> p l e", l=ct, e=EMBED)
            nc.vector.tensor_add(
                out=t2v, in0=g4[:, :ct, 0, :], in1=g4[:, :ct, 1, :]
            )
            tr = sb.tile([P, EMBED + 2], F32, tag="tr")
            nc.vector.tensor_reduce(
                out=tr[:, 0:EMBED],
                in_=t2[:, : EMBED * ct].rearrange("p (l e) -> p e l", l=ct, e=EMBED),
                axis=mybir.AxisListType.X,
                op=op.add,
            )
            nc.scalar.copy(
                out=tr[:, EMBED : EMBED + 2], in_=f_sb[:, 2 * t : 2 * t + 2]
            )

            # [P, 66] -> [66, P] so the projection contracts over E on partitions
            pT = ps.tile([EMBED + 2, P], F32, tag="pT", space="PSUM")
            nc.tensor.transpose(out=pT[:], in_=tr[:], identity=idt[:])
            mT = sb.tile([EMBED + 2, P], F32, tag="mT")
            nc.scalar.copy(out=mT[:], in_=pT[:])

            # out[128, 256] = meanT.T @ [W.T; b; null]: proj + bias + null select
            po = ps.tile([P, COND], F32, tag="po", space="PSUM")
            nc.tensor.matmul(out=po[:], lhsT=mT[:], rhs=wext_sb[:], start=True, stop=True)
            ob = sb.tile([P, COND], F32, tag="ob")
            nc.scalar.copy(out=ob[:], in_=po[:])
            nc.sync.dma_start(out=out[rows, :], in_=ob[:])

    nc.compile()
    return nc


_NC_CACHE: dict = {}


def _get_nc(cols: tuple) -> bass.Bass:
    if cols not in _NC_CACHE:
        _NC_CACHE[cols] = build_nc(cols)
    return _NC_CACHE[cols]


def _pack_embq(emb_table: np.ndarray) -> np.ndarray:
    emb_bf = np.asarray(emb_table, dtype=np.float32).astype(BF16_NP)  # [V, E]
    T = np.zeros((NSLOT, QROW), dtype=BF16_NP)
    T[1 : 1 + VOCAB // 4] = emb_bf.reshape(VOCAB // 4, QROW)
    return T


def prep(token_ids, lengths, emb_table, W, b, null_emb):
    """Returns (cols, in_maps, perm). Bags are sorted by length (descending)
    within each core; perm maps sorted row -> original row."""
    ids = np.asarray(token_ids).astype(np.int64, copy=False)  # [B, L]
    lens = np.asarray(lengths).astype(np.int64, copy=False)  # [B]

    # Sort bags per core by length descending (stable for determinism).
    perm = np.concatenate(
        [
            c * BLOC + np.argsort(-lens[c * BLOC : (c + 1) * BLOC], kind="stable")
            for c in range(NCORES)
        ]
    )
    ids = ids[perm]
    lens = lens[perm]

    # Per-tile column counts, maxed across cores so one SPMD program fits all.
    lt = lens.reshape(NCORES, NT, P)
    maxlen = lt.max(axis=2).max(axis=0)  # [NT]
    cols = tuple(int(m) for m in np.maximum(maxlen, 1))

    valid = np.arange(L)[None, :] < lens[:, None]  # [B, L]
    idx16 = np.where(valid, (ids >> 2) + 1, 0).astype(np.int16)  # [B, L]
    rec = (1.0 / np.maximum(lens, 1)).astype(np.float32)  # [B]
    sub = (ids & 3).astype(np.int64)  # [B, L]
    M = (
        (sub[:, :, None] == np.arange(4)[None, None, :]) & valid[:, :, None]
    ).astype(np.float32) * rec[:, None, None]  # [B, L, 4]
    M = M.astype(BF16_NP)
    fz = np.stack([(lens > 0), (lens == 0)], axis=1).astype(np.float32)  # [B, 2]

    embq = _pack_embq(emb_table)
    wext = np.concatenate(
        [
            np.asarray(W, dtype=np.float32).T,  # [64, 256]
            np.asarray(b, dtype=np.float32)[None, :],
            np.asarray(null_emb, dtype=np.float32)[None, :],
        ]
    )  # [66, 256]

    in_maps = []
    for c in range(NCORES):
        sl = slice(c * BLOC, (c + 1) * BLOC)
        A = idx16[sl].reshape(NT, P, L)  # [NT, P, L]
        Mc = M[sl].reshape(NT, P, L, 4)
        idx_parts, m_parts = [], []
        for t in range(NT):
            ct = cols[t]
            # idx stream: token (bag=t*128+p, l) at flat position i = l*128+p,
            # wrapped into 16 partitions (i%16, i//16), replicated to 128.
            At = A[t, :, :ct].T  # [ct, P]
            flat = At.reshape(ct * P)
            wrap = flat.reshape(ct * 8, 16).T  # [16, ct*8]
            idx_parts.append(np.tile(wrap, (8, 1)))  # [128, ct*8]
            m_parts.append(Mc[t, :, :ct, :].reshape(P, ct * 4))
        idxw = np.ascontiguousarray(np.concatenate(idx_parts, axis=1))
        mwc = np.ascontiguousarray(np.concatenate(m_parts, axis=1))
        Fc = fz[sl].reshape(NT, P, 2).transpose(1, 0, 2)
        fwc = np.ascontiguousarray(Fc.reshape(P, NT * 2))
        in_maps.append(
            {"embq": embq, "idxw": idxw, "mw": mwc, "fw": fwc, "wext": wext}
        )
    return cols, in_maps, perm


def make_in_maps(token_ids, lengths, emb_table, W, b, null_emb):
    return prep(token_ids, lengths, emb_table, W, b, null_emb)[1]


def kernel(token_ids, lengths, emb_table, W, b, null_emb, **run_kwargs):
    from concourse.bass_utils import run_bass_kernel_spmd

    cols, in_maps, perm = prep(token_ids, lengths, emb_table, W, b, null_emb)
    nc = _get_nc(cols)
    res = run_bass_kernel_spmd(nc, in_maps, core_ids=list(range(NCORES)), **run_kwargs)
    sorted_out = np.concatenate(
        [res.results[c]["out"] for c in range(NCORES)], axis=0
    )
    out = np.empty_like(sorted_out)
    out[perm] = sorted_out
    return out
